# revision 5
# baseline (speedup 1.0000x reference)
"""Trainium2 Bass kernel for linear attention over external memory.

Computes out = x @ (keys^T @ vals) for
  x [4, 2048, 1024] f32, keys/vals [65536, 1024] f32.

Sharding across 8 NeuronCores: keys/vals sharded along the memory dim M
(8192 rows per core); each core computes a partial kv = keys_s^T @ vals_s,
AllReduces kv in fp16 (split in two column halves), then computes its
token shard of x @ kv (x sharded by token, 1024 rows per core).

Stage 1 runs in float32r (TF32-like) directly on the DMA'd f32 data.
The PE is power-throttled to 13/16 * 2.4 GHz = 1.95 GHz under sustained
load, so 512-col matmuls issue every ~263 ns; stage 1 is clock-bound at
~269 us and the DMA stream (~243 GB/s demand) hides underneath it.

The 1 MiB fp16 AllReduces measure ~43 us (RDH) and serialize on the one
collective stream, so the tail is sized W=25: the last 25 chunks run
one column half at a time, h=0 from f32r staging (fires AR0 ~52 us
before stage-1 ends, fully hiding it under the h=1 pass), h=1 from an
fp16 hold cast on Vector/Scalar while the tail streams.  AR bounce
DMAs go out on the Scalar HWDGE queue in 4 chunks right behind the
PSUM drains.  x transposes + stage-2 h=0 fill most of the AR1 window;
a short warmer chain bridges the remaining PE idle so HAM doesn't
re-throttle the clock before stage-2 h=1.  Front groups are 6 chunks
(not 8) so the f32r staging pools shrink enough for the W=25 hold.
"""

import numpy as np

# Problem shapes (hardcoded per contract).
B, S, D = 4, 2048, 1024
M = 65536
NCORES = 8
P = 128
T = (B * S) // NCORES          # 1024 tokens per core
KM = M // NCORES               # 8192 memory rows per core
NC_ = KM // P                  # 64 k-chunks
DB = D // P                    # 8 d-blocks
HALF = D // 2                  # 512
TCH = T // P                   # 8 token chunks
W = 25                         # tail chunks (h-split, fp16 hold)
FRONT = NC_ - W                # 39
FRONT_GROUPS = [3, 6, 6, 6, 6, 6, 6]
assert sum(FRONT_GROUPS) == FRONT

_CACHE = {}


def _build_nc():
    import concourse.bacc as bacc
    import concourse.tile as tile
    from concourse import mybir
    from concourse.masks import make_identity

    f32 = mybir.dt.float32
    f32r = mybir.dt.float32r
    f16 = mybir.dt.float16
    ACT_COPY = mybir.ActivationFunctionType.Copy
    ADD = mybir.AluOpType.add

    nc = bacc.Bacc("TRN2", target_bir_lowering=False, debug=False,
                   num_devices=NCORES)

    xs_d = nc.dram_tensor("xs", [T, D], f32, kind="ExternalInput")
    ks_d = nc.dram_tensor("ks", [KM, D], f32r, kind="ExternalInput")
    vs_d = nc.dram_tensor("vs", [KM, D], f32r, kind="ExternalInput")
    out_d = nc.dram_tensor("out", [T, D], f32, kind="ExternalOutput")

    ks_r = ks_d.ap().rearrange("(c p) n -> c p n", p=P)   # [64, 128, 1024]
    vs_r = vs_d.ap().rearrange("(c p) n -> c p n", p=P)
    xs_r = xs_d.ap().rearrange("(c p) n -> c p n", p=P)   # [8, 128, 1024]

    with tile.TileContext(nc) as tc:
        with (
            tc.tile_pool(name="const", bufs=1) as const,
            tc.tile_pool(name="kfp", bufs=7) as kfp,
            tc.tile_pool(name="vfp", bufs=7) as vfp,
            tc.tile_pool(name="ktl", bufs=W) as ktl,
            tc.tile_pool(name="vtl", bufs=W) as vtl,
            tc.tile_pool(name="accp", bufs=2) as accp,
            tc.tile_pool(name="xstage", bufs=3) as xstage,
            tc.tile_pool(name="xtp", bufs=DB) as xtp,
            tc.tile_pool(name="kvio", bufs=2) as kvio,
            tc.tile_pool(name="outp", bufs=2) as outp,
            tc.tile_pool(name="ps", bufs=8, space="PSUM") as ps,
            tc.tile_pool(name="dram", bufs=8, space="DRAM") as dram,
        ):
            ident = const.tile([P, P], f32)
            make_identity(nc, ident)

            # Warm-up collective: arms the ncfw collective stream so the
            # first real AllReduce trigger doesn't pay the ~11us wake-up.
            # Kept tiny (4 KiB): a 1 MiB warmup was measured to stall the
            # front ~14us via HBM/link contention without making the real
            # RDH AllReduces any faster.
            warm = const.tile([P, 16], f16)
            nc.gpsimd.memset(warm[:], 0.0)
            warm_in = dram.tile([P, 16], f16, name="warm_in")
            warm_out = dram.tile([P, 16], f16, name="warm_out",
                                 addr_space="Shared")
            nc.gpsimd.dma_start(out=warm_in[:], in_=warm[:])
            nc.gpsimd.collective_compute(
                "AllReduce",
                ADD,
                replica_groups=[list(range(NCORES))],
                ins=[warm_in.opt()],
                outs=[warm_out.opt()],
            )

            # kv accumulator: acc[h][:, j*512:(j+1)*512] holds
            # kv[j*128:(j+1)*128, h*512:(h+1)*512] (fp16).
            acc = [accp.tile([P, DB * HALF], f16, name=f"acc{h}",
                             tag="acc") for h in range(2)]
            for h in range(2):
                nc.vector.memset(acc[h][:], 0.0)

            # ---- stage 1 front: 39 chunks in groups of 6, both halves --
            kf_tiles = [None] * NC_
            vf_tiles = [None] * NC_

            def load_chunk(c):
                kt = kfp.tile([P, D], f32r, name="kt", tag="kt")
                vt = vfp.tile([P, D], f32r, name="vt", tag="vt")
                if c < 2:
                    # Quarter-split so the first matmul's data lands ASAP
                    # after the DMA stream opens.
                    Q = D // 4
                    for q in range(4):
                        nc.sync.dma_start(out=kt[:, q * Q:(q + 1) * Q],
                                          in_=ks_r[c][:, q * Q:(q + 1) * Q])
                        nc.sync.dma_start(out=vt[:, q * Q:(q + 1) * Q],
                                          in_=vs_r[c][:, q * Q:(q + 1) * Q])
                else:
                    nc.sync.dma_start(out=kt[:], in_=ks_r[c])
                    nc.sync.dma_start(out=vt[:], in_=vs_r[c])
                kf_tiles[c] = kt
                vf_tiles[c] = vt

            c0 = 0
            for gi, gsz in enumerate(FRONT_GROUPS):
                for ci in range(gsz):
                    load_chunk(c0 + ci)
                for h in range(2):
                    e0 = h * HALF
                    pst = [ps.tile([P, HALF], f32, name=f"kv{h}_{j}",
                                   tag="ps") for j in range(DB)]
                    for ci in range(gsz):
                        c = c0 + ci
                        for j in range(DB):
                            nc.tensor.matmul(
                                pst[j][:],
                                kf_tiles[c][:, j * P:(j + 1) * P],
                                vf_tiles[c][:, e0:e0 + HALF],
                                start=(ci == 0), stop=(ci == gsz - 1))
                    for j in range(DB):
                        sl = slice(j * HALF, (j + 1) * HALF)
                        nc.vector.tensor_tensor(
                            out=acc[h][:, sl],
                            in0=pst[j][:],
                            in1=acc[h][:, sl],
                            op=ADD)
                c0 += gsz

            # Tail chunk loads continue the same stream.
            for c in range(FRONT, NC_):
                load_chunk(c)
            # x loads at the tail of the load stream (consumed by the
            # transposes after the tail, so a 3-deep pool pipelines fine).
            xf_tiles = []
            for i in range(TCH):
                xf = xstage.tile([P, D], f32, name="xf", tag="xf")
                nc.sync.dma_start(out=xf[:], in_=xs_r[i])
                xf_tiles.append(xf)

            # fp16 casts for the tail hold.  Emitted only now so they sit
            # behind all front drains in the Vector queue.  keys (full) on
            # Vector, vals' second half on Scalar.
            kh_tiles = [None] * NC_
            vh_tiles = [None] * NC_
            for c in range(FRONT, NC_):
                kh = ktl.tile([P, D], f16, name="kh", tag="kh")
                nc.vector.tensor_copy(out=kh[:],
                                      in_=kf_tiles[c][:].bitcast(f32))
                kh_tiles[c] = kh
            for c in range(FRONT, NC_):
                vh = vtl.tile([P, HALF], f16, name="vh", tag="vh")
                nc.scalar.activation(vh[:],
                                     vf_tiles[c][:, HALF:].bitcast(f32),
                                     ACT_COPY)
                vh_tiles[c] = vh

            def tail_drain(pst, h):
                for j in range(DB):
                    sl = slice(j * HALF, (j + 1) * HALF)
                    nc.vector.tensor_tensor(
                        out=acc[h][:, sl], in0=pst[j][:],
                        in1=acc[h][:, sl], op=ADD)

            # ---- stage 1 tail, h=0 from f32r staging ----
            pst = [ps.tile([P, HALF], f32, name=f"kvt0_{j}", tag="ps")
                   for j in range(DB)]
            for ci in range(W):
                c = FRONT + ci
                for j in range(DB):
                    nc.tensor.matmul(
                        pst[j][:],
                        kf_tiles[c][:, j * P:(j + 1) * P],
                        vf_tiles[c][:, 0:HALF],
                        start=(ci == 0), stop=(ci == W - 1))
            tail_drain(pst, 0)

            # AllReduce: bounce DMAs on the Scalar HWDGE queue in 4
            # two-slice chunks (pipelines behind the drains).
            bounce_out = []
            for h in range(2):
                b_in = dram.tile([P, DB * HALF], f16,
                                 name=f"bin{h}", tag="bin")
                b_out = dram.tile([P, DB * HALF], f16,
                                  name=f"bout{h}", tag="bout",
                                  addr_space="Shared")
                bounce_out.append((b_in, b_out))

            def emit_ar(h):
                b_in, b_out = bounce_out[h]
                for q in range(4):
                    sl = slice(q * 2 * HALF, (q + 1) * 2 * HALF)
                    nc.scalar.dma_start(out=b_in[:, sl], in_=acc[h][:, sl])
                nc.gpsimd.collective_compute(
                    "AllReduce",
                    ADD,
                    replica_groups=[list(range(NCORES))],
                    ins=[b_in.opt()],
                    outs=[b_out.opt()],
                )

            emit_ar(0)

            # ---- stage 1 tail, h=1 from the fp16 hold ----
            pst = [ps.tile([P, HALF], f32, name=f"kvt1_{j}", tag="ps")
                   for j in range(DB)]
            for ci in range(W):
                c = FRONT + ci
                for j in range(DB):
                    nc.tensor.matmul(
                        pst[j][:],
                        kh_tiles[c][:, j * P:(j + 1) * P],
                        vh_tiles[c][:],
                        start=(ci == 0), stop=(ci == W - 1))
            tail_drain(pst, 1)
            emit_ar(1)

            # ---- x: PE-transpose, cast to fp16 (fills AR1 wait) ----
            xT = [xtp.tile([P, T], f16, name=f"xT{j}", tag="xT")
                  for j in range(DB)]
            for i in range(TCH):
                xf = xf_tiles[i]
                for j in range(DB):
                    pst = ps.tile([P, P], f32, name="pst", tag="ps")
                    nc.tensor.transpose(
                        pst[:], xf[:, j * P:(j + 1) * P], ident[:])
                    nc.vector.tensor_copy(
                        out=xT[j][:, i * P:(i + 1) * P], in_=pst[:])

            # PE warmers: transposes + stage-2 h=0 fill ~37us of the
            # ~50us AR1 tail; the warmer chain bridges the rest so HAM
            # doesn't re-throttle the PE before stage-2 h=1.
            wsrc = const.tile([P, HALF], f32r)
            nc.vector.memset(wsrc[:].bitcast(f32), 0.0)

            def warmers(n):
                wps = ps.tile([P, HALF], f32, name="wps", tag="ps")
                for _ in range(n):
                    nc.tensor.matmul(wps[:], wsrc[:, :P], wsrc[:],
                                     start=True, stop=True)

            # ---- stage 2: out = x @ kv, per column half ----
            for h in range(2):
                kvh = kvio.tile([P, DB * HALF], f16, name=f"kvr{h}",
                                tag="kvio")
                # Split readback: stage-2's first i-chunk contracts j in
                # order, so it can start ~1.5us before the second half of
                # kv lands.
                nc.gpsimd.dma_start(out=kvh[:, :4 * HALF],
                                    in_=bounce_out[h][1][:, :4 * HALF])
                nc.gpsimd.dma_start(out=kvh[:, 4 * HALF:],
                                    in_=bounce_out[h][1][:, 4 * HALF:])
                if h == 1:
                    warmers(40)
                for i in range(TCH):
                    po = ps.tile([P, HALF], f32, name="po", tag="ps")
                    for j in range(DB):
                        nc.tensor.matmul(
                            po[:],
                            xT[j][:, i * P:(i + 1) * P],
                            kvh[:, j * HALF:(j + 1) * HALF],
                            start=(j == 0), stop=(j == DB - 1))
                    ob = outp.tile([P, HALF], f32, name="ob", tag="ob")
                    nc.scalar.activation(ob[:], po[:], ACT_COPY)
                    nc.scalar.dma_start(
                        out=out_d.ap()[i * P:(i + 1) * P,
                                       h * HALF:(h + 1) * HALF],
                        in_=ob[:])

    nc.compile()
    return nc


def _get_nc():
    if "nc" not in _CACHE:
        _CACHE["nc"] = _build_nc()
    return _CACHE["nc"]


def kernel(**inputs):
    from concourse.bass_utils import run_bass_kernel_spmd

    x = np.ascontiguousarray(np.asarray(inputs["x"], dtype=np.float32))
    keys = np.ascontiguousarray(np.asarray(inputs["keys"], dtype=np.float32))
    vals = np.ascontiguousarray(np.asarray(inputs["vals"], dtype=np.float32))
    xf = x.reshape(B * S, D)

    nc = _get_nc()
    in_maps = []
    for c in range(NCORES):
        in_maps.append({
            "xs": xf[c * T:(c + 1) * T],
            "ks": keys[c * KM:(c + 1) * KM],
            "vs": vals[c * KM:(c + 1) * KM],
        })
    res = run_bass_kernel_spmd(nc, in_maps, list(range(NCORES)))
    out = np.concatenate([res.results[c]["out"] for c in range(NCORES)],
                         axis=0)
    return out.reshape(B, S, D).astype(np.float32)


# revision 11
# speedup vs baseline: 1.1311x; 1.1311x over previous
"""Trainium2 Bass kernel for linear attention over external memory.

Computes out = x @ (keys^T @ vals) for
  x [4, 2048, 1024] f32, keys/vals [65536, 1024] f32.

Sharding across 8 NeuronCores: keys/vals sharded along the memory dim M
(8192 rows per core); each core computes a partial kv = keys_s^T @ vals_s,
AllReduces kv in fp16 (split in two column halves), then computes its
token shard of x @ kv (x sharded by token, 1024 rows per core).

Stage 1 runs in fp16: every chunk is DMA'd as f32, cast to fp16 (keys
on Vector, vals on Scalar) and matmul'd from the fp16 copy.  The PE is
power-throttled to 13/16 * 2.4 GHz under sustained load (512-col
matmuls issue every ~263 ns) so stage 1 is clock-bound at ~269 us; the
casts ride along on otherwise-idle engine time, and fp16 operands
halve SBUF traffic (a chance the power governor lifts the clock).

The 1 MiB fp16 AllReduces measure ~43 us (RDH) and serialize on the one
collective stream.  The tail (last W=23 chunks) is processed one column
half at a time so AR0 fires ~48 us before stage-1 ends, fully hidden
under the h=1 pass.  Crucially the tail chunks are loaded AND cast
during the front, using the front's spare DMA bandwidth (front demand
~243 GB/s of ~358): the tail phase is pure-PE, avoiding the 476 GB/s
DMA demand that stalls a load-as-you-go h-split tail.  AR bounce DMAs
go out on the Scalar HWDGE queue in 4 chunks right behind the PSUM
drains.  x transposes + stage-2 h=0 + a warmer chain fill the AR1
window so HAM doesn't re-throttle the PE before stage-2 h=1.
"""

import numpy as np

# Problem shapes (hardcoded per contract).
B, S, D = 4, 2048, 1024
M = 65536
NCORES = 8
P = 128
T = (B * S) // NCORES          # 1024 tokens per core
KM = M // NCORES               # 8192 memory rows per core
NC_ = KM // P                  # 64 k-chunks
DB = D // P                    # 8 d-blocks
HALF = D // 2                  # 512
TCH = T // P                   # 8 token chunks
W = 24                         # tail chunks (h-split)
FRONT = NC_ - W                # 40
FRONT_GROUPS = [8, 8, 8, 8, 8]
assert sum(FRONT_GROUPS) == FRONT
# Tail chunks produced (loaded+cast) per front group, riding the
# front's spare DMA bandwidth; the rest right after the last group.
TAIL_QUOTA = 2

_CACHE = {}


def _build_nc():
    import concourse.bacc as bacc
    import concourse.tile as tile
    from concourse import mybir
    from concourse.masks import make_identity

    f32 = mybir.dt.float32
    f32r = mybir.dt.float32r
    f16 = mybir.dt.float16
    ACT_COPY = mybir.ActivationFunctionType.Copy
    ADD = mybir.AluOpType.add

    nc = bacc.Bacc("TRN2", target_bir_lowering=False, debug=False,
                   num_devices=NCORES)

    xs_d = nc.dram_tensor("xs", [T, D], f32, kind="ExternalInput")
    ks_d = nc.dram_tensor("ks", [KM, D], f32, kind="ExternalInput")
    vs_d = nc.dram_tensor("vs", [KM, D], f32, kind="ExternalInput")
    out_d = nc.dram_tensor("out", [T, D], f32, kind="ExternalOutput")

    ks_r = ks_d.ap().rearrange("(c p) n -> c p n", p=P)   # [64, 128, 1024]
    vs_r = vs_d.ap().rearrange("(c p) n -> c p n", p=P)
    xs_r = xs_d.ap().rearrange("(c p) n -> c p n", p=P)   # [8, 128, 1024]

    with tile.TileContext(nc) as tc:
        with (
            tc.tile_pool(name="const", bufs=1) as const,
            tc.tile_pool(name="kstg", bufs=4) as kstg,
            tc.tile_pool(name="vstg", bufs=4) as vstg,
            tc.tile_pool(name="k16p", bufs=W + 2) as k16p,
            tc.tile_pool(name="v16p", bufs=W + 2) as v16p,
            tc.tile_pool(name="accp", bufs=2) as accp,
            tc.tile_pool(name="xstage", bufs=4) as xstage,
            tc.tile_pool(name="xtp", bufs=DB) as xtp,
            tc.tile_pool(name="kvio", bufs=2) as kvio,
            tc.tile_pool(name="outp", bufs=2) as outp,
            tc.tile_pool(name="ps", bufs=8, space="PSUM") as ps,
            tc.tile_pool(name="dram", bufs=8, space="DRAM") as dram,
        ):
            ident = const.tile([P, P], f32)
            make_identity(nc, ident)

            # Warm-up collective: arms the ncfw collective stream so the
            # first real AllReduce trigger doesn't pay the ~11us wake-up.
            # Kept tiny: a 1 MiB warmup stalls the front via HBM/link
            # contention without making the real RDH AllReduces faster.
            warm = const.tile([P, 16], f16)
            nc.gpsimd.memset(warm[:], 0.0)
            warm_in = dram.tile([P, 16], f16, name="warm_in")
            warm_out = dram.tile([P, 16], f16, name="warm_out",
                                 addr_space="Shared")
            nc.gpsimd.dma_start(out=warm_in[:], in_=warm[:])
            nc.gpsimd.collective_compute(
                "AllReduce",
                ADD,
                replica_groups=[list(range(NCORES))],
                ins=[warm_in.opt()],
                outs=[warm_out.opt()],
            )

            # kv accumulator: acc[h][:, j*512:(j+1)*512] holds
            # kv[j*128:(j+1)*128, h*512:(h+1)*512] (fp16).
            acc = [accp.tile([P, DB * HALF], f16, name=f"acc{h}",
                             tag="acc") for h in range(2)]
            for h in range(2):
                nc.vector.memset(acc[h][:], 0.0)

            # Warmer source, zeroed up front while the Vector queue is
            # otherwise waiting on the first input DMAs.
            wsrc = const.tile([P, HALF], f32r)
            nc.vector.memset(wsrc[:].bitcast(f32), 0.0)

            k16 = [None] * NC_
            v16 = [None] * NC_

            def produce(c, cast_eng_k):
                """Load chunk c (f32) and cast to fp16.

                Front chunks cast keys on Vector / vals on Scalar.  Tail
                chunks cast both on Scalar (cast_eng_k=scalar): their
                DMAs land late in each group window and a Vector-queue
                cast waiting on them would head-block the PSUM drains.
                """
                kt = kstg.tile([P, D], f32, name="kt", tag="kt")
                vt = vstg.tile([P, D], f32, name="vt", tag="vt")
                kc = k16p.tile([P, D], f16, name="k16", tag="k16")
                vc = v16p.tile([P, D], f16, name="v16", tag="v16")
                if c < 2:
                    # Half-split loads AND casts: the h=0 matmuls need
                    # the full k but only v's first half, and each cast
                    # waits only its own half's DMA, so the first matmul
                    # issues ~6us sooner than with whole-tile produce.
                    for s in (slice(0, HALF), slice(HALF, D)):
                        nc.sync.dma_start(out=kt[:, s], in_=ks_r[c][:, s])
                        nc.sync.dma_start(out=vt[:, s], in_=vs_r[c][:, s])
                        nc.vector.tensor_copy(out=kc[:, s], in_=kt[:, s])
                        nc.scalar.activation(vc[:, s], vt[:, s], ACT_COPY)
                else:
                    nc.sync.dma_start(out=kt[:], in_=ks_r[c])
                    nc.sync.dma_start(out=vt[:], in_=vs_r[c])
                    if cast_eng_k == "vector":
                        nc.vector.tensor_copy(out=kc[:], in_=kt[:])
                    else:
                        nc.scalar.activation(kc[:], kt[:], ACT_COPY)
                    nc.scalar.activation(vc[:], vt[:], ACT_COPY)
                k16[c] = kc
                v16[c] = vc

            def front_group(c0, gsz, h):
                e0 = h * HALF
                pst = [ps.tile([P, HALF], f32, name=f"kv{h}_{j}",
                               tag="ps") for j in range(DB)]
                for ci in range(gsz):
                    c = c0 + ci
                    for j in range(DB):
                        nc.tensor.matmul(
                            pst[j][:],
                            k16[c][:, j * P:(j + 1) * P],
                            v16[c][:, e0:e0 + HALF],
                            start=(ci == 0), stop=(ci == gsz - 1))
                for j in range(DB):
                    sl = slice(j * HALF, (j + 1) * HALF)
                    nc.vector.tensor_tensor(
                        out=acc[h][:, sl],
                        in0=pst[j][:],
                        in1=acc[h][:, sl],
                        op=ADD)

            # ---- stage 1 front, software-pipelined produce ----
            next_tail = FRONT
            for ci in range(FRONT_GROUPS[0]):
                produce(ci, "vector")
            c0 = 0
            for gi, gsz in enumerate(FRONT_GROUPS):
                nxt = c0 + gsz
                if gi + 1 < len(FRONT_GROUPS):
                    for ci in range(FRONT_GROUPS[gi + 1]):
                        produce(nxt + ci, "vector")
                front_group(c0, gsz, 0)
                # Tail produce after the h0 drains are queued so their
                # late-landing DMAs can't head-block the Vector queue.
                for _ in range(TAIL_QUOTA):
                    if next_tail < NC_:
                        produce(next_tail, "scalar")
                        next_tail += 1
                front_group(c0, gsz, 1)
                c0 = nxt
            while next_tail < NC_:
                produce(next_tail, "scalar")
                next_tail += 1

            # x loads at the tail of the load stream (consumed by the
            # transposes after the tail; the tail phase is DMA-idle).
            xf_tiles = []
            for i in range(TCH):
                xf = xstage.tile([P, D], f32, name="xf", tag="xf")
                nc.sync.dma_start(out=xf[:], in_=xs_r[i])
                xf_tiles.append(xf)

            def tail_drain(pst, h):
                for j in range(DB):
                    sl = slice(j * HALF, (j + 1) * HALF)
                    nc.vector.tensor_tensor(
                        out=acc[h][:, sl], in0=pst[j][:],
                        in1=acc[h][:, sl], op=ADD)

            # ---- stage 1 tail, h=0 ----
            pst = [ps.tile([P, HALF], f32, name=f"kvt0_{j}", tag="ps")
                   for j in range(DB)]
            for ci in range(W):
                c = FRONT + ci
                for j in range(DB):
                    nc.tensor.matmul(
                        pst[j][:],
                        k16[c][:, j * P:(j + 1) * P],
                        v16[c][:, 0:HALF],
                        start=(ci == 0), stop=(ci == W - 1))
            tail_drain(pst, 0)

            # AllReduce: bounce DMAs on the Scalar HWDGE queue in 4
            # two-slice chunks (pipelines behind the drains).
            bounce_out = []
            for h in range(2):
                b_in = dram.tile([P, DB * HALF], f16,
                                 name=f"bin{h}", tag="bin")
                b_out = dram.tile([P, DB * HALF], f16,
                                  name=f"bout{h}", tag="bout",
                                  addr_space="Shared")
                bounce_out.append((b_in, b_out))

            def emit_ar(h):
                b_in, b_out = bounce_out[h]
                for q in range(4):
                    sl = slice(q * 2 * HALF, (q + 1) * 2 * HALF)
                    nc.scalar.dma_start(out=b_in[:, sl], in_=acc[h][:, sl])
                nc.gpsimd.collective_compute(
                    "AllReduce",
                    ADD,
                    replica_groups=[list(range(NCORES))],
                    ins=[b_in.opt()],
                    outs=[b_out.opt()],
                )

            emit_ar(0)

            # ---- stage 1 tail, h=1 ----
            pst = [ps.tile([P, HALF], f32, name=f"kvt1_{j}", tag="ps")
                   for j in range(DB)]
            for ci in range(W):
                c = FRONT + ci
                for j in range(DB):
                    nc.tensor.matmul(
                        pst[j][:],
                        k16[c][:, j * P:(j + 1) * P],
                        v16[c][:, HALF:],
                        start=(ci == 0), stop=(ci == W - 1))
            tail_drain(pst, 1)
            emit_ar(1)

            # ---- x: PE-transpose, cast to fp16 (fills AR1 wait) ----
            xT = [xtp.tile([P, T], f16, name=f"xT{j}", tag="xT")
                  for j in range(DB)]
            for i in range(TCH):
                xf = xf_tiles[i]
                for j in range(DB):
                    pst = ps.tile([P, P], f32, name="pst", tag="ps")
                    nc.tensor.transpose(
                        pst[:], xf[:, j * P:(j + 1) * P], ident[:])
                    nc.vector.tensor_copy(
                        out=xT[j][:, i * P:(i + 1) * P], in_=pst[:])

            # PE warmers: transposes + stage-2 h=0 fill ~36us of the
            # ~48us AR1 tail; the warmer chain bridges the rest so HAM
            # doesn't re-throttle the PE before stage-2 h=1.
            def warmers(n):
                wps = ps.tile([P, HALF], f32, name="wps", tag="ps")
                for _ in range(n):
                    nc.tensor.matmul(wps[:], wsrc[:, :P], wsrc[:],
                                     start=True, stop=True)

            # ---- stage 2: out = x @ kv, per column half ----
            for h in range(2):
                kvh = kvio.tile([P, DB * HALF], f16, name=f"kvr{h}",
                                tag="kvio")
                # Split readback: stage-2's first i-chunk contracts j in
                # order, so it can start before the second half lands.
                nc.gpsimd.dma_start(out=kvh[:, :4 * HALF],
                                    in_=bounce_out[h][1][:, :4 * HALF])
                nc.gpsimd.dma_start(out=kvh[:, 4 * HALF:],
                                    in_=bounce_out[h][1][:, 4 * HALF:])
                if h == 1:
                    warmers(50)
                for i in range(TCH):
                    po = ps.tile([P, HALF], f32, name="po", tag="ps")
                    for j in range(DB):
                        nc.tensor.matmul(
                            po[:],
                            xT[j][:, i * P:(i + 1) * P],
                            kvh[:, j * HALF:(j + 1) * HALF],
                            start=(j == 0), stop=(j == DB - 1))
                    ob = outp.tile([P, HALF], f32, name="ob", tag="ob")
                    nc.scalar.activation(ob[:], po[:], ACT_COPY)
                    nc.scalar.dma_start(
                        out=out_d.ap()[i * P:(i + 1) * P,
                                       h * HALF:(h + 1) * HALF],
                        in_=ob[:])

    nc.compile()
    return nc


def _get_nc():
    if "nc" not in _CACHE:
        _CACHE["nc"] = _build_nc()
    return _CACHE["nc"]


def kernel(**inputs):
    from concourse.bass_utils import run_bass_kernel_spmd

    x = np.ascontiguousarray(np.asarray(inputs["x"], dtype=np.float32))
    keys = np.ascontiguousarray(np.asarray(inputs["keys"], dtype=np.float32))
    vals = np.ascontiguousarray(np.asarray(inputs["vals"], dtype=np.float32))
    xf = x.reshape(B * S, D)

    nc = _get_nc()
    in_maps = []
    for c in range(NCORES):
        in_maps.append({
            "xs": xf[c * T:(c + 1) * T],
            "ks": keys[c * KM:(c + 1) * KM],
            "vs": vals[c * KM:(c + 1) * KM],
        })
    res = run_bass_kernel_spmd(nc, in_maps, list(range(NCORES)))
    out = np.concatenate([res.results[c]["out"] for c in range(NCORES)],
                         axis=0)
    return out.reshape(B, S, D).astype(np.float32)
